# revision 16
# baseline (speedup 1.0000x reference)
"""MCR2 loss kernel for 8 Trainium2 NeuronCores.

Class-sorted data-parallel sharding: the host permutes samples so each
class occupies a contiguous, zero-padded block of CAP rows, then
pre-transposes each core's shard into the partition-major device layout
[128 partitions, groups*128 bytes] (fp8), so every DMA descriptor is
one contiguous partition line and transfers can be sliced at any
512-row group boundary.

Device work per core: stream the 2.4 MB fp8 shard in progressively
sized transfers (1/2/4/8/16/24/... groups of 512 rows) alternating
across the two HWDGE queues; for each 4-tile group run one
[128,128]x[128,128] fp8 matmul whose four diagonal 32x32 PSUM blocks
are the per-tile Grams; accumulate 30 groups per PSUM region (one
region per 15360-row chunk, each chunk inside one class block); copy
each region to SBUF and DMA out.  Warmup matmuls on a zeroed tile
bridge the ~2.5us from kernel start to first data so the PE HAM
clock-gate un-throttles (1.2 -> 2.4 GHz) as early as possible; the
real stream is kept gapless for the same reason.  Host evaluates the
32x32 logdets in float64.
"""

import sys

sys.path.insert(0, "/opt/trn_rl_repo")

import numpy as np

import concourse.bacc as bacc
import concourse.bass as bass  # noqa: F401
import concourse.mybir as mybir
import concourse.tile as tile
from concourse.bass_utils import run_bass_kernel_spmd

N, D, C = 600000, 32, 10
EPS = 0.5
NCORES = 8
CAP = 61440                      # padded rows per class block (~60000 + 6 sigma)
ROWS_PER_CORE = C * CAP // NCORES        # 76800
GROUPS_PER_CORE = ROWS_PER_CORE // 512   # 150
TILES_PER_CORE = GROUPS_PER_CORE * 4     # 600

# DMA transfers: (queue, n_groups) in consumption order.  Tiny whole
# transfers first (the ~2.5us HWDGE pipeline + completion-receipt
# latency is roughly size-independent, so small first transfers let the
# PE start early); the rest as pairs split across both queues so each
# span is delivered at aggregate ring bandwidth and completes in
# consumption order.  queue 0 = sync, 1 = scalar.
# Span sizes in groups.  Each span is delivered as two parallel
# transfers: sync carries partitions 0-63, scalar 64-127 (64
# descriptors each, even/odd SDMA-ring sets), so descriptor emission
# for a span is half as long and both queues work every span.
XFER_GROUPS = [6, 10, 12, 14, 16, 18, 20, 24, 30]
assert sum(XFER_GROUPS) == GROUPS_PER_CORE

# PSUM accumulation regions: 30 groups = 15360 rows per chunk; every
# chunk lies inside one class block (CAP = 4 chunks).
CHUNK_GROUPS = 30
NCHUNK = GROUPS_PER_CORE // CHUNK_GROUPS         # 5
CHUNKS_PER_CLASS = CAP // (CHUNK_GROUPS * 512)   # 4

WARMUPS = 28                     # bridge kernel-start -> first data (~3.0us)

_cache = {}


def _build_program():
    nc = bacc.Bacc(None)
    f8 = mybir.dt.float8e4
    f32 = mybir.dt.float32
    # Partition-major shard: partition p holds byte [g*128 + t*32 + d] for
    # row g*512 + t*128 + p of the (sorted, padded) shard.
    z_dram = nc.dram_tensor("Z", [128, GROUPS_PER_CORE * 128], f8,
                            kind="ExternalInput")
    out_dram = nc.dram_tensor("grams", [128, NCHUNK * 128], f32,
                              kind="ExternalOutput")

    with tile.TileContext(nc) as tc:
        with (
            tc.tile_pool(name="z", bufs=len(XFER_GROUPS)) as z_pool,
            tc.tile_pool(name="outp", bufs=1) as out_pool,
            tc.tile_pool(name="psum", bufs=NCHUNK, space="PSUM") as psum_pool,
            tc.tile_pool(name="warm", bufs=1) as warm_pool,
            tc.tile_pool(name="wpsum", bufs=1, space="PSUM") as wpsum_pool,
        ):
            # Input transfers, issued up front.  Each span is its own SBUF
            # tile so matmul groups wait only on the transfers that carry
            # them; partition halves go out on both queues in parallel.
            z_tiles = []     # (tile, start_group, n_groups)
            off = 0
            for k, ng in enumerate(XFER_GROUPS):
                z_sb = z_pool.tile([128, ng * 128], f8, tag=f"z{k}")
                src = z_dram[:, off * 128:(off + ng) * 128]
                nc.sync.dma_start(z_sb[0:64, :], src[0:64, :])
                nc.scalar.dma_start(z_sb[64:128, :], src[64:128, :])
                z_tiles.append((z_sb, off, ng))
                off += ng

            # Warmup matmuls: keep the PE busy from kernel start until the
            # first transfer lands, so the HAM activity window fills and
            # the clock un-throttles to 2.4 GHz as early as possible.
            warm_z = warm_pool.tile([128, 128], f8)
            nc.vector.memset(warm_z[:], 0.0)
            wacc = wpsum_pool.tile([128, 128], f32)
            for _ in range(WARMUPS):
                nc.tensor.matmul(wacc[:], warm_z[:], warm_z[:], start=True, stop=True)

            # Real stream: one matmul per 4-tile group, accumulated into the
            # chunk's PSUM region; extract regions to SBUF as chunks finish.
            out_sb = out_pool.tile([128, NCHUNK * 128], f32)
            xfer_idx = 0
            acc = None
            for g in range(GROUPS_PER_CORE):
                while g >= z_tiles[xfer_idx][1] + z_tiles[xfer_idx][2]:
                    xfer_idx += 1
                z_sb, a, _ = z_tiles[xfer_idx]
                zg = z_sb[:, (g - a) * 128:(g - a + 1) * 128]
                c = g // CHUNK_GROUPS
                if g % CHUNK_GROUPS == 0:
                    acc = psum_pool.tile([128, 128], f32, tag="acc")
                nc.tensor.matmul(
                    acc[:], zg, zg,
                    start=(g % CHUNK_GROUPS == 0),
                    stop=(g % CHUNK_GROUPS == CHUNK_GROUPS - 1),
                )
                if g % CHUNK_GROUPS == CHUNK_GROUPS - 1:
                    dst = out_sb[:, c * 128:(c + 1) * 128]
                    if c < NCHUNK - 1:
                        nc.vector.tensor_copy(dst, acc[:])
                    else:
                        # split the tail copy across DVE and ACT
                        nc.vector.tensor_copy(dst[:, 0:64], acc[:, 0:64])
                        nc.scalar.copy(dst[:, 64:128], acc[:, 64:128])
                    if c == NCHUNK - 2:
                        nc.scalar.dma_start(
                            out_dram[:, :(NCHUNK - 1) * 128],
                            out_sb[:, :(NCHUNK - 1) * 128],
                        )
            # Final region halved across both queues to shorten the tail.
            lo = (NCHUNK - 1) * 128
            mid = lo + 64
            nc.sync.dma_start(out_dram[:, lo:mid], out_sb[:, lo:mid])
            nc.scalar.dma_start(out_dram[:, mid:], out_sb[:, mid:])

    nc.compile()
    return nc


def kernel(Z: np.ndarray, labels: np.ndarray) -> np.ndarray:
    Z = np.asarray(Z, dtype=np.float32)
    labels = np.asarray(labels, dtype=np.int32)

    if "nc" not in _cache:
        _cache["nc"] = _build_program()
    nc = _cache["nc"]

    counts = np.bincount(labels, minlength=C)
    order = np.argsort(labels, kind="stable")

    f8np = mybir.dt.np(mybir.dt.float8e4)
    Zp = np.zeros([C * CAP, D], f8np)
    host_extra = np.zeros([C, D, D], np.float64)
    off = 0
    for j in range(C):
        cnt = int(counts[j])
        take = min(cnt, CAP)
        Zp[j * CAP:j * CAP + take] = Z[order[off:off + take]]
        if cnt > CAP:
            extra = Z[order[off + take:off + cnt]].astype(np.float64)
            host_extra[j] = extra.T @ extra
        off += cnt

    # Pre-transpose each core's shard to the partition-major device layout:
    # dev[p, g*128 + t*32 + d] = shard[g*512 + t*128 + p, d]
    in_maps = []
    for k in range(NCORES):
        shard = Zp[k * ROWS_PER_CORE:(k + 1) * ROWS_PER_CORE]
        dev = np.ascontiguousarray(
            shard.reshape(TILES_PER_CORE, 128, D).transpose(1, 0, 2)
        ).reshape(128, TILES_PER_CORE * D)
        in_maps.append({"Z": dev})

    res = run_bass_kernel_spmd(nc, in_maps, core_ids=list(range(NCORES)))
    _cache["last_results"] = res

    gj = host_extra.copy()
    for k, r in enumerate(res.results):
        # [128, 5*128]: per chunk a [128,128] region whose four diagonal
        # 32x32 blocks are per-tile-slot Gram sums; chunk -> class.
        g = r["grams"].astype(np.float64).reshape(128, NCHUNK, 128)
        for c in range(NCHUNK):
            cls = (k * NCHUNK + c) // CHUNKS_PER_CLASS
            for b in range(4):
                gj[cls] += g[b * D:(b + 1) * D, c, b * D:(b + 1) * D]

    g_all = gj.sum(axis=0)
    tr_pi = counts.astype(np.float64)

    nf, df = float(N), float(D)
    eye = np.eye(D)
    loss_r = 0.5 * np.linalg.slogdet(eye + (df / (nf * EPS)) * g_all)[1]
    loss_rc = 0.0
    for j in range(C):
        ld = np.linalg.slogdet(eye + (df / (tr_pi[j] * EPS)) * gj[j])[1]
        loss_rc += (tr_pi[j] / (2.0 * nf)) * ld
    loss_obj = loss_r - loss_rc
    return np.asarray([-loss_obj, loss_r, loss_rc], dtype=np.float32)


# revision 20
# speedup vs baseline: 1.0136x; 1.0136x over previous
"""MCR2 loss kernel for 8 Trainium2 NeuronCores.

Class-sorted data-parallel sharding: the host permutes samples so each
class occupies a contiguous, zero-padded block of CAP rows, then
pre-transposes each core's shard into the partition-major device layout
[128 partitions, groups*128 bytes] (fp8), so every DMA descriptor is
one contiguous partition line and transfers can be sliced at any
512-row group boundary.

Device work per core: stream the 2.4 MB fp8 shard in progressively
sized transfers (1/2/4/8/16/24/... groups of 512 rows) alternating
across the two HWDGE queues; for each 4-tile group run one
[128,128]x[128,128] fp8 matmul whose four diagonal 32x32 PSUM blocks
are the per-tile Grams; accumulate 30 groups per PSUM region (one
region per 15360-row chunk, each chunk inside one class block); copy
each region to SBUF and DMA out.  Warmup matmuls on a zeroed tile
bridge the ~2.5us from kernel start to first data so the PE HAM
clock-gate un-throttles (1.2 -> 2.4 GHz) as early as possible; the
real stream is kept gapless for the same reason.  Host evaluates the
32x32 logdets in float64.
"""

import sys

sys.path.insert(0, "/opt/trn_rl_repo")

import numpy as np

import concourse.bacc as bacc
import concourse.bass as bass  # noqa: F401
import concourse.mybir as mybir
import concourse.tile as tile
from concourse.bass_utils import run_bass_kernel_spmd

N, D, C = 600000, 32, 10
EPS = 0.5
NCORES = 8
CAP = 61440                      # padded rows per class block (~60000 + 6 sigma)
ROWS_PER_CORE = C * CAP // NCORES        # 76800
GROUPS_PER_CORE = ROWS_PER_CORE // 512   # 150
TILES_PER_CORE = GROUPS_PER_CORE * 4     # 600

# DMA transfers: (queue, n_groups) in consumption order.  Tiny whole
# transfers first (the ~2.5us HWDGE pipeline + completion-receipt
# latency is roughly size-independent, so small first transfers let the
# PE start early); the rest as pairs split across both queues so each
# span is delivered at aggregate ring bandwidth and completes in
# consumption order.  queue 0 = sync, 1 = scalar.
# Span sizes in groups.  Each span is delivered as two parallel
# transfers: sync carries partitions 0-63, scalar 64-127 (64
# descriptors each, even/odd SDMA-ring sets), so descriptor emission
# for a span is half as long and both queues work every span.
XFER_GROUPS = [8, 10, 12, 14, 16, 18, 20, 24, 28]
assert sum(XFER_GROUPS) == GROUPS_PER_CORE

# PSUM accumulation regions: 30 groups = 15360 rows per chunk; every
# chunk lies inside one class block (CAP = 4 chunks).
CHUNK_GROUPS = 30
NCHUNK = GROUPS_PER_CORE // CHUNK_GROUPS         # 5
CHUNKS_PER_CLASS = CAP // (CHUNK_GROUPS * 512)   # 4

WARMUPS = 28                     # bridge kernel-start -> first data (~3.0us)

_cache = {}


def _build_program():
    nc = bacc.Bacc(None)
    f8 = mybir.dt.float8e4
    f32 = mybir.dt.float32
    # Partition-major shard: partition p holds byte [g*128 + t*32 + d] for
    # row g*512 + t*128 + p of the (sorted, padded) shard.
    z_dram = nc.dram_tensor("Z", [128, GROUPS_PER_CORE * 128], f8,
                            kind="ExternalInput")
    out_dram = nc.dram_tensor("grams", [128, NCHUNK * 128], f32,
                              kind="ExternalOutput")

    # Warmup operand zeroed during the framework preamble (GpSimd main
    # block, before the all-engine barrier) so the PE's first warmup
    # matmul has no intra-body dependency and dispatches immediately at
    # kernel-body start.
    warm_raw = nc.alloc_sbuf_tensor("warm_z_pre", [128, 128], f8)
    nc.gpsimd.memset(warm_raw.ap(), 0.0)

    with tile.TileContext(nc) as tc:
        with (
            tc.tile_pool(name="z", bufs=len(XFER_GROUPS)) as z_pool,
            tc.tile_pool(name="outp", bufs=1) as out_pool,
            tc.tile_pool(name="psum", bufs=NCHUNK, space="PSUM") as psum_pool,
            tc.tile_pool(name="wpsum", bufs=1, space="PSUM") as wpsum_pool,
        ):
            # Input transfers, issued up front.  Each span is its own SBUF
            # tile so matmul groups wait only on the transfers that carry
            # them; partition halves go out on both queues in parallel.
            z_tiles = []     # (tile, start_group, n_groups)
            off = 0
            for k, ng in enumerate(XFER_GROUPS):
                z_sb = z_pool.tile([128, ng * 128], f8, tag=f"z{k}")
                src = z_dram[:, off * 128:(off + ng) * 128]
                nc.sync.dma_start(z_sb[0:64, :], src[0:64, :])
                nc.scalar.dma_start(z_sb[64:128, :], src[64:128, :])
                z_tiles.append((z_sb, off, ng))
                off += ng

            # Warmup matmuls: keep the PE busy from kernel start until the
            # first transfer lands, so the HAM activity window fills and
            # the clock un-throttles to 2.4 GHz as early as possible.
            warm_z = warm_raw.ap()
            wacc = wpsum_pool.tile([128, 128], f32)
            for _ in range(WARMUPS):
                nc.tensor.matmul(wacc[:], warm_z, warm_z, start=True, stop=True)

            # Real stream: one matmul per 4-tile group, accumulated into the
            # chunk's PSUM region; extract regions to SBUF as chunks finish.
            out_sb = out_pool.tile([128, NCHUNK * 128], f32)
            xfer_idx = 0
            acc = None
            for g in range(GROUPS_PER_CORE):
                while g >= z_tiles[xfer_idx][1] + z_tiles[xfer_idx][2]:
                    xfer_idx += 1
                z_sb, a, _ = z_tiles[xfer_idx]
                zg = z_sb[:, (g - a) * 128:(g - a + 1) * 128]
                c = g // CHUNK_GROUPS
                if g % CHUNK_GROUPS == 0:
                    acc = psum_pool.tile([128, 128], f32, tag="acc")
                nc.tensor.matmul(
                    acc[:], zg, zg,
                    start=(g % CHUNK_GROUPS == 0),
                    stop=(g % CHUNK_GROUPS == CHUNK_GROUPS - 1),
                )
                if g % CHUNK_GROUPS == CHUNK_GROUPS - 1:
                    dst = out_sb[:, c * 128:(c + 1) * 128]
                    if c < NCHUNK - 1:
                        nc.vector.tensor_copy(dst, acc[:])
                    else:
                        # split the tail copy across DVE and ACT
                        nc.vector.tensor_copy(dst[:, 0:64], acc[:, 0:64])
                        nc.scalar.copy(dst[:, 64:128], acc[:, 64:128])
                    if c == NCHUNK - 2:
                        nc.scalar.dma_start(
                            out_dram[:, :(NCHUNK - 1) * 128],
                            out_sb[:, :(NCHUNK - 1) * 128],
                        )
            # Final region halved across both queues to shorten the tail.
            lo = (NCHUNK - 1) * 128
            mid = lo + 64
            nc.sync.dma_start(out_dram[:, lo:mid], out_sb[:, lo:mid])
            nc.scalar.dma_start(out_dram[:, mid:], out_sb[:, mid:])

    nc.compile()
    return nc


def kernel(Z: np.ndarray, labels: np.ndarray) -> np.ndarray:
    Z = np.asarray(Z, dtype=np.float32)
    labels = np.asarray(labels, dtype=np.int32)

    if "nc" not in _cache:
        _cache["nc"] = _build_program()
    nc = _cache["nc"]

    counts = np.bincount(labels, minlength=C)
    order = np.argsort(labels, kind="stable")

    f8np = mybir.dt.np(mybir.dt.float8e4)
    Zp = np.zeros([C * CAP, D], f8np)
    host_extra = np.zeros([C, D, D], np.float64)
    off = 0
    for j in range(C):
        cnt = int(counts[j])
        take = min(cnt, CAP)
        Zp[j * CAP:j * CAP + take] = Z[order[off:off + take]]
        if cnt > CAP:
            extra = Z[order[off + take:off + cnt]].astype(np.float64)
            host_extra[j] = extra.T @ extra
        off += cnt

    # Pre-transpose each core's shard to the partition-major device layout:
    # dev[p, g*128 + t*32 + d] = shard[g*512 + t*128 + p, d]
    in_maps = []
    for k in range(NCORES):
        shard = Zp[k * ROWS_PER_CORE:(k + 1) * ROWS_PER_CORE]
        dev = np.ascontiguousarray(
            shard.reshape(TILES_PER_CORE, 128, D).transpose(1, 0, 2)
        ).reshape(128, TILES_PER_CORE * D)
        in_maps.append({"Z": dev})

    res = run_bass_kernel_spmd(nc, in_maps, core_ids=list(range(NCORES)))
    _cache["last_results"] = res

    gj = host_extra.copy()
    for k, r in enumerate(res.results):
        # [128, 5*128]: per chunk a [128,128] region whose four diagonal
        # 32x32 blocks are per-tile-slot Gram sums; chunk -> class.
        g = r["grams"].astype(np.float64).reshape(128, NCHUNK, 128)
        for c in range(NCHUNK):
            cls = (k * NCHUNK + c) // CHUNKS_PER_CLASS
            for b in range(4):
                gj[cls] += g[b * D:(b + 1) * D, c, b * D:(b + 1) * D]

    g_all = gj.sum(axis=0)
    tr_pi = counts.astype(np.float64)

    nf, df = float(N), float(D)
    eye = np.eye(D)
    loss_r = 0.5 * np.linalg.slogdet(eye + (df / (nf * EPS)) * g_all)[1]
    loss_rc = 0.0
    for j in range(C):
        ld = np.linalg.slogdet(eye + (df / (tr_pi[j] * EPS)) * gj[j])[1]
        loss_rc += (tr_pi[j] / (2.0 * nf)) * ld
    loss_obj = loss_r - loss_rc
    return np.asarray([-loss_obj, loss_r, loss_rc], dtype=np.float32)
